# revision 7
# baseline (speedup 1.0000x reference)
"""Trainium2 Bass kernel for nn_Attention (cumulative masked softmax attention).

Reference computation:
    v   = tanh(x @ W + b)                  (B, T, F)
    a   = v . u                            (B, T)   -- query-independent logits
    e   = exp(a)[:, None, :] * tril * mask (B, T, T)
    alf = e / (sum_s e + EPS)
    c   = alf @ x                          (B, T, F)

Because the logits are query-independent and the mask is lower-triangular,
the (B,T,T) softmax-matmul collapses to a running weighted average:
    w[s]  = exp(a[s]) * mask[s]
    c[t]  = cumsum_s(w * x)[t] / (cumsum_s(w)[t] + EPS)
which is O(B*T*F) instead of O(B*T^2*F).

Sharding: data-parallel over batch B across 8 NeuronCores (2 batches/core).
W/u/b replicated. Each core processes 2048 rows of (T, F); the cumsum is
done per-batch with triangular/ones matmul blocks on the tensor engine.
Matmul operands use float32r (fp32 with ~12-bit mantissa, full PE rate).
"""

import numpy as np

import concourse.bass as bass  # noqa: F401
import concourse.tile as tile
from concourse import bacc, mybir
from concourse.bass_utils import run_bass_kernel_spmd
from concourse.masks import make_identity, make_upper_triangular

B, T, F = 16, 1024, 512
EPS = 1e-7
NCORES = 8
B_LOC = B // NCORES          # batches per core
R = B_LOC * T                # rows per core
P = 128                      # partition tile
NT = R // P                  # row tiles per core
NTB = T // P                 # row tiles per batch
KC = F // P                  # contraction chunks

F32 = mybir.dt.float32
F32R = mybir.dt.float32r


def _build(have_b: bool, have_mask: bool, mm_f32r=True, scan_f32r=True,
           repeat: int = 1):
    nc = bacc.Bacc("TRN2", target_bir_lowering=False, debug=False)

    mm_dt = F32R if mm_f32r else F32
    sc_dt = F32R if scan_f32r else F32

    x_d = nc.dram_tensor("x", [NT, P, F], F32, kind="ExternalInput")
    w_d = nc.dram_tensor("W", [KC, P, F], F32, kind="ExternalInput")
    u_d = nc.dram_tensor("u", [1, F], F32, kind="ExternalInput")
    if have_b:
        b_d = nc.dram_tensor("b", [1, F], F32, kind="ExternalInput")
    if have_mask:
        m_d = nc.dram_tensor("m", [NT, P, 1], F32, kind="ExternalInput")
    c_d = nc.dram_tensor("c", [NT, P, F], F32, kind="ExternalOutput")

    Tanh = mybir.ActivationFunctionType.Tanh
    Exp = mybir.ActivationFunctionType.Exp
    Copy = mybir.ActivationFunctionType.Copy
    ADD = mybir.AluOpType.add

    with tile.TileContext(nc) as tc:
        with (
            tc.tile_pool(name="const", bufs=1) as const,
            tc.tile_pool(name="xp", bufs=3) as xp,
            tc.tile_pool(name="xtp", bufs=3) as xtp,
            tc.tile_pool(name="vp", bufs=2) as vp,
            tc.tile_pool(name="scrp", bufs=2) as scrp,
            tc.tile_pool(name="yp", bufs=NT) as yp,
            tc.tile_pool(name="wp", bufs=NT) as wp,
            tc.tile_pool(name="smal", bufs=4) as smal,
            tc.tile_pool(name="cp", bufs=3) as cp,
            tc.tile_pool(name="ps_tr", bufs=2, space="PSUM") as ps_tr_pool,
            tc.tile_pool(name="ps_v", bufs=2, space="PSUM") as ps_v_pool,
            tc.tile_pool(name="ps_P", bufs=2, space="PSUM") as ps_P_pool,
            tc.tile_pool(name="ps_Z", bufs=2, space="PSUM") as ps_Z_pool,
        ):
            # ---- constants ----
            W_sb = const.tile([P, KC, F], mm_dt)
            for k in range(KC):
                wf = scrp.tile([P, F], F32, tag="wstage")
                nc.sync.dma_start(out=wf, in_=w_d.ap()[k])
                nc.vector.tensor_copy(W_sb[:, k, :], wf)
            u_bc = const.tile([P, F], F32)
            nc.gpsimd.dma_start(out=u_bc, in_=u_d.ap().to_broadcast((P, F)))
            if have_b:
                b_sb = const.tile([1, F], mm_dt)
                bf = smal.tile([1, F], F32, tag="bstage")
                nc.sync.dma_start(out=bf, in_=b_d.ap())
                nc.vector.tensor_copy(b_sb, bf)
                ones_row = const.tile([1, P], mm_dt)
                nc.vector.memset(ones_row, 1.0)
            triu_f = const.tile([P, P], F32)
            make_upper_triangular(nc, triu_f, val=1.0, diag=True)
            triu = const.tile([P, P], sc_dt)
            nc.vector.tensor_copy(triu, triu_f)
            ones = const.tile([P, P], sc_dt)
            onesf = const.tile([P, P], F32)
            nc.vector.memset(onesf, 1.0)
            nc.vector.tensor_copy(ones, onesf)
            ident = const.tile([P, P], F32)
            make_identity(nc, ident)

            for _rep in range(repeat):
                ys = []
                ws = []
                # ---- phase A: per-row weights w and weighted values y ----
                for i in range(NT):
                    xt = xp.tile([P, F], F32)
                    nc.sync.dma_start(out=xt, in_=x_d.ap()[i])

                    ps_tr = ps_tr_pool.tile([P, F], F32)
                    for k in range(KC):
                        nc.tensor.transpose(
                            ps_tr[:, k * P:(k + 1) * P],
                            xt[:, k * P:(k + 1) * P],
                            ident,
                        )
                    xT = xtp.tile([P, F], mm_dt)
                    nc.scalar.copy(out=xT, in_=ps_tr)

                    ps_v = ps_v_pool.tile([P, F], F32)
                    for k in range(KC):
                        nc.tensor.matmul(
                            ps_v,
                            xT[:, k * P:(k + 1) * P],
                            W_sb[:, k, :],
                            start=(k == 0),
                            stop=(k == KC - 1 and not have_b),
                        )
                    if have_b:
                        nc.tensor.matmul(
                            ps_v, ones_row, b_sb, start=False, stop=True
                        )

                    v = vp.tile([P, F], F32)
                    nc.scalar.activation(out=v, in_=ps_v, func=Tanh)

                    scr = scrp.tile([P, F], F32)
                    alpha = smal.tile([P, 1], F32)
                    nc.vector.tensor_mul(scr, v, u_bc)
                    nc.vector.tensor_reduce(alpha, scr, axis=mybir.AxisListType.X,
                                            op=ADD)

                    w = smal.tile([P, 1], F32)
                    nc.scalar.activation(out=w, in_=alpha, func=Exp)
                    if have_mask:
                        mt = smal.tile([P, 1], F32)
                        nc.sync.dma_start(out=mt, in_=m_d.ap()[i])
                        nc.vector.tensor_mul(w, w, mt)
                    # fp32r matmuls need an even innermost free count, so the
                    # Z-cumsum operand is duplicated into two columns.
                    wr = wp.tile([P, 2], sc_dt)
                    nc.vector.tensor_copy(wr[:, 0:1], w)
                    nc.vector.tensor_copy(wr[:, 1:2], w)

                    y = yp.tile([P, F], sc_dt)
                    nc.vector.tensor_scalar_mul(y, xt, w)
                    ys.append(y)
                    ws.append(wr)

                # ---- phase B: blockwise cumsum via triangular matmuls ----
                for i in range(NT):
                    ib = i % NTB          # tile index within batch
                    base = i - ib         # first tile of this batch
                    ps_P = ps_P_pool.tile([P, F], F32)
                    ps_Z = ps_Z_pool.tile([P, 2], F32)
                    nc.tensor.matmul(
                        ps_P, triu, ys[i], start=True, stop=(ib == 0)
                    )
                    nc.tensor.matmul(
                        ps_Z, triu, ws[i], start=True, stop=(ib == 0)
                    )
                    for j in range(ib):
                        nc.tensor.matmul(
                            ps_P, ones, ys[base + j],
                            start=False, stop=(j == ib - 1),
                        )
                        nc.tensor.matmul(
                            ps_Z, ones, ws[base + j],
                            start=False, stop=(j == ib - 1),
                        )

                    zr = smal.tile([P, 1], F32)
                    nc.vector.tensor_scalar_add(zr, ps_Z[:, 0:1], EPS)
                    rec = smal.tile([P, 1], F32)
                    nc.vector.reciprocal(rec, zr)

                    c = cp.tile([P, F], F32)
                    nc.scalar.activation(out=c, in_=ps_P, func=Copy, scale=rec)
                    nc.sync.dma_start(out=c_d.ap()[i], in_=c)

    nc.compile()
    return nc


_NC_CACHE: dict = {}


def _get_nc(have_b, have_mask, mm_f32r=True, scan_f32r=True, repeat=1):
    key = (have_b, have_mask, mm_f32r, scan_f32r, repeat)
    if key not in _NC_CACHE:
        _NC_CACHE[key] = _build(have_b, have_mask, mm_f32r, scan_f32r, repeat)
    return _NC_CACHE[key]


def kernel(x, mask, W, b, u):
    x = np.ascontiguousarray(np.asarray(x, dtype=np.float32))
    W = np.ascontiguousarray(np.asarray(W, dtype=np.float32))
    b = np.asarray(b, dtype=np.float32)
    u = np.asarray(u, dtype=np.float32)
    mask_f = np.asarray(mask).astype(np.float32)

    have_b = bool(np.any(b != 0.0))
    have_mask = bool(np.any(mask_f != 1.0))

    nc = _get_nc(have_b, have_mask)

    W_r = W.reshape(KC, P, F)
    u_r = u.reshape(1, F)
    in_maps = []
    for core in range(NCORES):
        xs = x[core * B_LOC:(core + 1) * B_LOC].reshape(NT, P, F)
        m = {"x": xs, "W": W_r, "u": u_r}
        if have_b:
            m["b"] = b.reshape(1, F)
        if have_mask:
            m["m"] = mask_f[core * B_LOC:(core + 1) * B_LOC].reshape(NT, P, 1)
        in_maps.append(m)

    res = run_bass_kernel_spmd(nc, in_maps, core_ids=list(range(NCORES)))
    out = np.stack([r["c"].reshape(B_LOC, T, F) for r in res.results])
    return out.reshape(B, T, F)


# revision 10
# speedup vs baseline: 901.3233x; 901.3233x over previous
"""Trainium2 Bass kernel for nn_Attention (cumulative masked softmax attention).

Reference computation:
    v   = tanh(x @ W + b)                  (B, T, F)
    a   = v . u                            (B, T)   -- query-independent logits
    e   = exp(a)[:, None, :] * tril * mask (B, T, T)
    alf = e / (sum_s e + EPS)
    c   = alf @ x                          (B, T, F)

Because the logits are query-independent and the mask is lower-triangular,
the (B,T,T) softmax-matmul collapses to a running weighted average:
    w[s]  = exp(a[s]) * mask[s]
    c[t]  = cumsum_s(w * x)[t] / (cumsum_s(w)[t] + EPS)
which is O(B*T*F) instead of O(B*T^2*F).

Sharding: data-parallel over batch B across 8 NeuronCores (2 batches/core).
W/u/b replicated. Each core processes 2048 rows of (T, F); the cumsum is
done per-batch with triangular/ones matmul blocks on the tensor engine.
Matmul operands use float32r (fp32 with ~12-bit mantissa, full PE rate).
"""

import numpy as np

import concourse.bass as bass  # noqa: F401
import concourse.tile as tile
from concourse import bacc, mybir
from concourse.bass_utils import run_bass_kernel_spmd
from concourse.masks import make_identity, make_upper_triangular

B, T, F = 16, 1024, 512
EPS = 1e-7
NCORES = 8
B_LOC = B // NCORES          # batches per core
R = B_LOC * T                # rows per core
P = 128                      # partition tile
NT = R // P                  # row tiles per core
NTB = T // P                 # row tiles per batch
KC = F // P                  # contraction chunks

F32 = mybir.dt.float32
F32R = mybir.dt.float32r


def _build(have_b: bool, have_mask: bool, mm_f32r=True, scan_f32r=True,
           repeat: int = 1, loop_n: int = 0):
    """loop_n > 0 wraps the body in a hardware For_i loop (timing only)."""
    nc = bacc.Bacc("TRN2", target_bir_lowering=False, debug=False)

    mm_dt = F32R if mm_f32r else F32
    sc_dt = F32R if scan_f32r else F32

    x_d = nc.dram_tensor("x", [NT, P, F], F32, kind="ExternalInput")
    w_d = nc.dram_tensor("W", [KC, P, F], F32, kind="ExternalInput")
    u_d = nc.dram_tensor("u", [1, F], F32, kind="ExternalInput")
    if have_b:
        b_d = nc.dram_tensor("b", [1, F], F32, kind="ExternalInput")
    if have_mask:
        m_d = nc.dram_tensor("m", [NT, P, 1], F32, kind="ExternalInput")
    c_d = nc.dram_tensor("c", [NT, P, F], F32, kind="ExternalOutput")

    Tanh = mybir.ActivationFunctionType.Tanh
    Exp = mybir.ActivationFunctionType.Exp
    Copy = mybir.ActivationFunctionType.Copy
    ADD = mybir.AluOpType.add

    with tile.TileContext(nc) as tc:
        with (
            tc.tile_pool(name="const", bufs=1) as const,
            tc.tile_pool(name="xp", bufs=3) as xp,
            tc.tile_pool(name="xtp", bufs=3) as xtp,
            tc.tile_pool(name="vp", bufs=2) as vp,
            tc.tile_pool(name="scrp", bufs=2) as scrp,
            tc.tile_pool(name="yp", bufs=NT) as yp,
            tc.tile_pool(name="wp", bufs=NT) as wp,
            tc.tile_pool(name="smal", bufs=4) as smal,
            tc.tile_pool(name="cp", bufs=3) as cp,
            tc.tile_pool(name="ps_tr", bufs=2, space="PSUM") as ps_tr_pool,
            tc.tile_pool(name="ps_v", bufs=2, space="PSUM") as ps_v_pool,
            tc.tile_pool(name="ps_P", bufs=2, space="PSUM") as ps_P_pool,
            tc.tile_pool(name="ps_Z", bufs=2, space="PSUM") as ps_Z_pool,
        ):
            # ---- constants ----
            W_sb = const.tile([P, KC, F], mm_dt)
            for k in range(KC):
                wf = scrp.tile([P, F], F32, tag="wstage")
                nc.sync.dma_start(out=wf, in_=w_d.ap()[k])
                nc.vector.tensor_copy(W_sb[:, k, :], wf)
            u_bc = const.tile([P, F], F32)
            nc.gpsimd.dma_start(out=u_bc, in_=u_d.ap().to_broadcast((P, F)))
            if have_b:
                b_sb = const.tile([1, F], mm_dt)
                bf = smal.tile([1, F], F32, tag="bstage")
                nc.sync.dma_start(out=bf, in_=b_d.ap())
                nc.vector.tensor_copy(b_sb, bf)
                ones_row = const.tile([1, P], mm_dt)
                nc.vector.memset(ones_row, 1.0)
            triu_f = const.tile([P, P], F32)
            make_upper_triangular(nc, triu_f, val=1.0, diag=True)
            triu = const.tile([P, P], sc_dt)
            nc.vector.tensor_copy(triu, triu_f)
            ones = const.tile([P, P], sc_dt)
            onesf = const.tile([P, P], F32)
            nc.vector.memset(onesf, 1.0)
            nc.vector.tensor_copy(ones, onesf)
            ident = const.tile([P, P], F32)
            make_identity(nc, ident)

            import contextlib
            loop_ctx = (tc.For_i(0, loop_n, 1) if loop_n
                        else contextlib.nullcontext())
            with loop_ctx:
              for _rep in range(repeat):
                ys = []
                ws = []
                # ---- phase A: per-row weights w and weighted values y ----
                for i in range(NT):
                    xt = xp.tile([P, F], F32)
                    nc.sync.dma_start(out=xt, in_=x_d.ap()[i])

                    ps_tr = ps_tr_pool.tile([P, F], F32)
                    for k in range(KC):
                        nc.tensor.transpose(
                            ps_tr[:, k * P:(k + 1) * P],
                            xt[:, k * P:(k + 1) * P],
                            ident,
                        )
                    xT = xtp.tile([P, F], mm_dt)
                    nc.scalar.copy(out=xT, in_=ps_tr)

                    ps_v = ps_v_pool.tile([P, F], F32)
                    for k in range(KC):
                        nc.tensor.matmul(
                            ps_v,
                            xT[:, k * P:(k + 1) * P],
                            W_sb[:, k, :],
                            start=(k == 0),
                            stop=(k == KC - 1 and not have_b),
                        )
                    if have_b:
                        nc.tensor.matmul(
                            ps_v, ones_row, b_sb, start=False, stop=True
                        )

                    v = vp.tile([P, F], F32)
                    nc.scalar.activation(out=v, in_=ps_v, func=Tanh)

                    scr = scrp.tile([P, F], F32)
                    alpha = smal.tile([P, 1], F32)
                    nc.vector.tensor_mul(scr, v, u_bc)
                    nc.vector.tensor_reduce(alpha, scr, axis=mybir.AxisListType.X,
                                            op=ADD)

                    w = smal.tile([P, 1], F32)
                    nc.scalar.activation(out=w, in_=alpha, func=Exp)
                    if have_mask:
                        mt = smal.tile([P, 1], F32)
                        nc.sync.dma_start(out=mt, in_=m_d.ap()[i])
                        nc.vector.tensor_mul(w, w, mt)
                    # fp32r matmuls need an even innermost free count, so the
                    # Z-cumsum operand is duplicated into two columns.
                    wr = wp.tile([P, 2], sc_dt)
                    nc.vector.tensor_copy(wr[:, 0:1], w)
                    nc.vector.tensor_copy(wr[:, 1:2], w)

                    y = yp.tile([P, F], sc_dt)
                    nc.vector.tensor_scalar_mul(y, xt, w)
                    ys.append(y)
                    ws.append(wr)

                # ---- phase B: blockwise cumsum via triangular matmuls ----
                for i in range(NT):
                    ib = i % NTB          # tile index within batch
                    base = i - ib         # first tile of this batch
                    ps_P = ps_P_pool.tile([P, F], F32)
                    ps_Z = ps_Z_pool.tile([P, 2], F32)
                    nc.tensor.matmul(
                        ps_P, triu, ys[i], start=True, stop=(ib == 0)
                    )
                    nc.tensor.matmul(
                        ps_Z, triu, ws[i], start=True, stop=(ib == 0)
                    )
                    for j in range(ib):
                        nc.tensor.matmul(
                            ps_P, ones, ys[base + j],
                            start=False, stop=(j == ib - 1),
                        )
                        nc.tensor.matmul(
                            ps_Z, ones, ws[base + j],
                            start=False, stop=(j == ib - 1),
                        )

                    zr = smal.tile([P, 1], F32)
                    nc.vector.tensor_scalar_add(zr, ps_Z[:, 0:1], EPS)
                    rec = smal.tile([P, 1], F32)
                    nc.vector.reciprocal(rec, zr)

                    c = cp.tile([P, F], F32)
                    nc.scalar.activation(out=c, in_=ps_P, func=Copy, scale=rec)
                    nc.sync.dma_start(out=c_d.ap()[i], in_=c)

    nc.compile()
    return nc


_NC_CACHE: dict = {}


def _get_nc(have_b, have_mask, mm_f32r=True, scan_f32r=True, repeat=1,
            loop_n=0):
    key = (have_b, have_mask, mm_f32r, scan_f32r, repeat, loop_n)
    if key not in _NC_CACHE:
        _NC_CACHE[key] = _build(have_b, have_mask, mm_f32r, scan_f32r, repeat,
                                loop_n)
    return _NC_CACHE[key]


def kernel(x, mask, W, b, u):
    x = np.ascontiguousarray(np.asarray(x, dtype=np.float32))
    W = np.ascontiguousarray(np.asarray(W, dtype=np.float32))
    b = np.asarray(b, dtype=np.float32)
    u = np.asarray(u, dtype=np.float32)
    mask_f = np.asarray(mask).astype(np.float32)

    have_b = bool(np.any(b != 0.0))
    have_mask = bool(np.any(mask_f != 1.0))

    nc = _get_nc(have_b, have_mask)

    W_r = W.reshape(KC, P, F)
    u_r = u.reshape(1, F)
    in_maps = []
    for core in range(NCORES):
        xs = x[core * B_LOC:(core + 1) * B_LOC].reshape(NT, P, F)
        m = {"x": xs, "W": W_r, "u": u_r}
        if have_b:
            m["b"] = b.reshape(1, F)
        if have_mask:
            m["m"] = mask_f[core * B_LOC:(core + 1) * B_LOC].reshape(NT, P, 1)
        in_maps.append(m)

    res = run_bass_kernel_spmd(nc, in_maps, core_ids=list(range(NCORES)))
    out = np.stack([r["c"].reshape(B_LOC, T, F) for r in res.results])
    return out.reshape(B, T, F)


# revision 22
# speedup vs baseline: 1066.9646x; 1.1838x over previous
"""Trainium2 Bass kernel for nn_Attention (cumulative masked softmax attention).

Reference computation:
    v   = tanh(x @ W + b)                  (B, T, F)
    a   = v . u                            (B, T)   -- query-independent logits
    e   = exp(a)[:, None, :] * tril * mask (B, T, T)
    alf = e / (sum_s e + EPS)
    c   = alf @ x                          (B, T, F)

Because the logits are query-independent and the mask is lower-triangular,
the (B,T,T) softmax-matmul collapses to a running weighted average:
    w[s]  = exp(a[s]) * mask[s]
    c[t]  = cumsum_s(w * x)[t] / (cumsum_s(w)[t] + EPS)
which is O(B*T*F) instead of O(B*T^2*F).

Sharding: data-parallel over batch B across 8 NeuronCores (2 batches/core).
W/u/b replicated. Each core processes 2048 rows of (T, F); the (w*x) cumsum
is done per-batch with triangular/ones matmul blocks on the tensor engine,
and the scalar cumsum of w via two tiny matmuls + a free-dim prefix scan.
Matmul operands use float32r (fp32, ~11-bit mantissa, full PE rate).
The host supplies x both in natural layout and pre-transposed (xT) so the
tensor engine needs no on-chip transposes for the x @ W contraction.
"""

import numpy as np

import concourse.bass as bass  # noqa: F401
import concourse.tile as tile
from concourse import bacc, mybir
from concourse.bass_utils import run_bass_kernel_spmd

B, T, F = 16, 1024, 512
EPS = 1e-7
NCORES = 8
B_LOC = B // NCORES          # batches per core
R = B_LOC * T                # rows per core
P = 128                      # partition tile
NT = R // P                  # row tiles per core
NTB = T // P                 # row tiles per batch
KC = F // P                  # contraction chunks

F32 = mybir.dt.float32
F32R = mybir.dt.float32r


def _build(have_b: bool, have_mask: bool, loop_n: int = 0):
    """Build the per-core Bass module. loop_n > 0 wraps the body in a
    hardware For_i loop (used only for timing)."""
    nc = bacc.Bacc("TRN2", target_bir_lowering=False, debug=False)

    x_d = nc.dram_tensor("x", [NT, P, F], F32, kind="ExternalInput")
    xt_d = nc.dram_tensor("xT", [NT, P, F], F32R, kind="ExternalInput")
    w_d = nc.dram_tensor("W", [KC, P, F], F32, kind="ExternalInput")
    u_d = nc.dram_tensor("u", [1, F], F32, kind="ExternalInput")
    if have_b:
        b_d = nc.dram_tensor("b", [1, F], F32, kind="ExternalInput")
    if have_mask:
        m_d = nc.dram_tensor("m", [NT, P, 1], F32, kind="ExternalInput")
    c_d = nc.dram_tensor("c", [NT, P, F], F32, kind="ExternalOutput")

    Tanh = mybir.ActivationFunctionType.Tanh
    Exp = mybir.ActivationFunctionType.Exp
    Copy = mybir.ActivationFunctionType.Copy
    ADD = mybir.AluOpType.add
    SUB = mybir.AluOpType.subtract

    NP = NT // 2             # tile pairs

    with tile.TileContext(nc) as tc:
        with (
            tc.tile_pool(name="const", bufs=1) as const,
            tc.tile_pool(name="xp", bufs=3) as xp,
            tc.tile_pool(name="xtp", bufs=3) as xtp,
            tc.tile_pool(name="vp", bufs=2) as vp,
            tc.tile_pool(name="scrp", bufs=2) as scrp,
            tc.tile_pool(name="yp", bufs=NT) as yp,
            tc.tile_pool(name="wap", bufs=B_LOC) as wap,
            tc.tile_pool(name="smal", bufs=6) as smal,
            tc.tile_pool(name="cp", bufs=3) as cp,
            tc.tile_pool(name="ps_v", bufs=2, space="PSUM") as ps_v_pool,
            tc.tile_pool(name="ps_P", bufs=2, space="PSUM") as ps_P_pool,
            tc.tile_pool(name="ps_Z", bufs=1, space="PSUM") as ps_Z_pool,
        ):
            # ---- constants ----
            W_sb = const.tile([P, KC, F], F32R)
            for k in range(KC):
                wf = scrp.tile([P, F], F32, tag="wstage")
                nc.sync.dma_start(out=wf, in_=w_d.ap()[k])
                nc.vector.tensor_copy(W_sb[:, k, :], wf)
            u_bc2 = const.tile([P, 2, F], F32)
            nc.gpsimd.dma_start(out=u_bc2[:, 0, :],
                                in_=u_d.ap().to_broadcast((P, F)))
            nc.gpsimd.dma_start(out=u_bc2[:, 1, :],
                                in_=u_d.ap().to_broadcast((P, F)))
            if have_b:
                b_sb = const.tile([1, F], F32R)
                bf = smal.tile([1, F], F32, tag="bstage")
                nc.sync.dma_start(out=bf, in_=b_d.ap())
                nc.vector.tensor_copy(b_sb, bf)
                ones_row = const.tile([1, P], F32R)
                nc.vector.memset(ones_row, 1.0)
            # triangular + ones matmul weights for the cumsum (exact in f32r)
            triu_f = const.tile([P, P], F32)
            nc.gpsimd.memset(triu_f, 0.0)
            nc.gpsimd.affine_select(
                out=triu_f, in_=triu_f, compare_op=mybir.AluOpType.is_gt,
                fill=1.0, base=0, pattern=[[-1, P]], channel_multiplier=1)
            triu = const.tile([P, P], F32R)
            nc.vector.tensor_copy(triu, triu_f)
            ones = const.tile([P, P], F32R)
            onesf = const.tile([P, P], F32)
            nc.vector.memset(onesf, 1.0)
            nc.vector.tensor_copy(ones, onesf)
            zeros8 = const.tile([P, NTB], F32)
            nc.vector.memset(zeros8, 0.0)

            import contextlib
            loop_ctx = (tc.For_i(0, loop_n, 1) if loop_n
                        else contextlib.nullcontext())
            with loop_ctx:
              for batch in range(B_LOC):
                ys = []
                # ---- phase A: logits -> weights w, weighted values y ----
                w_all = wap.tile([P, NTB], F32)
                for pp in range(NTB // 2):
                    i0 = batch * NTB + 2 * pp

                    ps_v2 = ps_v_pool.tile([P, 2, F], F32)
                    xT2 = xtp.tile([P, 2, F], F32R)
                    nc.sync.dma_start(
                        out=xT2,
                        in_=xt_d.ap()[i0:i0 + 2].rearrange("j p f -> p j f"))
                    xt2 = xp.tile([P, 2, F], F32)
                    nc.sync.dma_start(
                        out=xt2,
                        in_=x_d.ap()[i0:i0 + 2].rearrange("j p f -> p j f"))
                    for j in range(2):
                        for k in range(KC):
                            nc.tensor.matmul(
                                ps_v2[:, j, :],
                                xT2[:, j, k * P:(k + 1) * P],
                                W_sb[:, k, :],
                                start=(k == 0),
                                stop=(k == KC - 1 and not have_b),
                            )
                        if have_b:
                            nc.tensor.matmul(ps_v2[:, j, :], ones_row, b_sb,
                                             start=False, stop=True)
                    xts = [xt2[:, 0, :], xt2[:, 1, :]]

                    v2 = vp.tile([P, 2, F], F32)
                    nc.scalar.activation(out=v2, in_=ps_v2, func=Tanh)
                    scr2 = scrp.tile([P, 2, F], F32)
                    nc.gpsimd.tensor_mul(scr2, v2, u_bc2)
                    alpha2 = smal.tile([P, 2], F32)
                    nc.vector.tensor_reduce(alpha2, scr2,
                                            axis=mybir.AxisListType.X, op=ADD)
                    ib0 = i0 % NTB
                    nc.scalar.activation(out=w_all[:, ib0:ib0 + 2],
                                         in_=alpha2, func=Exp)
                    if have_mask:
                        mt = smal.tile([P, 2], F32)
                        for j in range(2):
                            nc.sync.dma_start(out=mt[:, j:j + 1],
                                              in_=m_d.ap()[i0 + j])
                        nc.vector.tensor_mul(w_all[:, ib0:ib0 + 2],
                                             w_all[:, ib0:ib0 + 2], mt)
                    for j in range(2):
                        y = yp.tile([P, F], F32R)
                        nc.vector.tensor_scalar_mul(
                            y, xts[j], w_all[:, ib0 + j:ib0 + j + 1])
                        ys.append(y)

                # ---- phase B: Z prefixes then blockwise cumsum ----
                wr_all = wap.tile([P, NTB], F32R, tag="wr")
                nc.vector.tensor_copy(wr_all, w_all)
                ps_A = ps_Z_pool.tile([P, NTB], F32)
                ps_B = ps_Z_pool.tile([P, NTB], F32)
                nc.tensor.matmul(ps_A, triu, wr_all, start=True, stop=True)
                nc.tensor.matmul(ps_B, ones, wr_all, start=True, stop=True)
                S = smal.tile([P, NTB], F32)
                nc.vector.tensor_tensor_scan(
                    out=S, data0=ps_B, data1=zeros8, initial=0.0,
                    op0=ADD, op1=ADD)
                D = smal.tile([P, NTB], F32)
                nc.vector.tensor_tensor(out=D, in0=S, in1=ps_B, op=SUB)
                Z = smal.tile([P, NTB], F32)
                nc.vector.tensor_tensor(out=Z, in0=D, in1=ps_A, op=ADD)
                zr = smal.tile([P, NTB], F32)
                nc.vector.tensor_scalar_add(zr, Z, EPS)
                rec = smal.tile([P, NTB], F32, tag="rec")
                nc.vector.reciprocal(rec, zr)

                cs = []
                for ib in range(NTB):
                    i = batch * NTB + ib
                    ps_P = ps_P_pool.tile([P, F], F32)
                    nc.tensor.matmul(ps_P, triu, ys[ib], start=True,
                                     stop=(ib == 0))
                    for j in range(ib):
                        nc.tensor.matmul(ps_P, ones, ys[j],
                                         start=False, stop=(j == ib - 1))
                    if ib % 2 == 0:
                        c2 = cp.tile([P, 2, F], F32)
                        cs.append(c2)
                    nc.scalar.activation(out=cs[-1][:, ib % 2, :], in_=ps_P,
                                         func=Copy,
                                         scale=rec[:, ib:ib + 1])
                    if ib % 2 == 1:
                        nc.scalar.dma_start(
                            out=c_d.ap()[i - 1:i + 1].rearrange(
                                "j p f -> p j f"),
                            in_=cs[-1])

    nc.compile()
    return nc


_NC_CACHE: dict = {}


def _get_nc(have_b, have_mask, loop_n=0):
    key = (have_b, have_mask, loop_n)
    if key not in _NC_CACHE:
        _NC_CACHE[key] = _build(have_b, have_mask, loop_n)
    return _NC_CACHE[key]


def _host_xt(xs):
    """xs: (NT, P, F) tile-major core shard -> pre-transposed layout where
    xt[i, p, k*128+t] = xs[i, t, k*128+p] (chunk-transposed for matmul lhsT)."""
    v = xs.reshape(NT, P, KC, P).transpose(0, 3, 2, 1)
    return np.ascontiguousarray(v).reshape(NT, P, F)


def make_core_maps(x, W, u, b=None, mask_f=None):
    """Build the 8 per-core input maps from full inputs."""
    W_r = np.ascontiguousarray(W.reshape(KC, P, F))
    u_r = np.ascontiguousarray(u.reshape(1, F))
    maps = []
    for core in range(NCORES):
        xs = np.ascontiguousarray(
            x[core * B_LOC:(core + 1) * B_LOC].reshape(NT, P, F))
        m = {"x": xs, "xT": _host_xt(xs), "W": W_r, "u": u_r}
        if b is not None:
            m["b"] = np.ascontiguousarray(b.reshape(1, F))
        if mask_f is not None:
            m["m"] = np.ascontiguousarray(
                mask_f[core * B_LOC:(core + 1) * B_LOC].reshape(NT, P, 1))
        maps.append(m)
    return maps


def kernel(x, mask, W, b, u):
    x = np.asarray(x, dtype=np.float32)
    W = np.asarray(W, dtype=np.float32)
    b = np.asarray(b, dtype=np.float32)
    u = np.asarray(u, dtype=np.float32)
    mask_f = np.asarray(mask).astype(np.float32)

    have_b = bool(np.any(b != 0.0))
    have_mask = bool(np.any(mask_f != 1.0))

    nc = _get_nc(have_b, have_mask)
    in_maps = make_core_maps(x, W, u,
                             b if have_b else None,
                             mask_f if have_mask else None)
    res = run_bass_kernel_spmd(nc, in_maps, core_ids=list(range(NCORES)))
    out = np.stack([r["c"].reshape(B_LOC, T, F) for r in res.results])
    return out.reshape(B, T, F)
